# revision 20
# baseline (speedup 1.0000x reference)
"""Trainium2 Bass kernel for DiffeomorphicTransform (scaling-and-squaring).

flow_0 = velocity / 2^7; 7x: flow += trilinear_sample(flow, grid + flow)

Strategy (8 NeuronCores, SPMD):
  - Shard: batch (2) x z-slab (4) -> each core owns ZS=40 z-slices of one batch.
  - Gather source: replicated per-batch "A" volume in fp16, channels-last with
    z-pair AND y-pair duplication: A[z][y][x][zz][yy][c] = flow[z+zz, y+yy, x, c]
    (edge-clamped). All 24 trilinear corner values for a voxel base (z0,y0,x0)
    are one contiguous 48-byte run -> ONE indirect-DMA descriptor per voxel,
    128 voxels per gather instruction.
  - A volume (incl. A_0) is built on device from flow shards; interior
    shard-boundary slices are repaired after each AllGather.
  - Inter-core: AllGather of fp16 A-shards within each 4-core batch group.
"""

import sys

for _p in ("/opt/trn_rl_repo",):
    if _p not in sys.path:
        sys.path.append(_p)

import numpy as np
import concourse.bass as bass
import concourse.mybir as mybir
import concourse.tile as tile
from concourse.bass import AP
from concourse.bass_utils import run_bass_kernel_spmd

F32 = mybir.dt.float32
F16 = mybir.dt.float16
I32 = mybir.dt.int32
OP = mybir.AluOpType

TIME_STEP = 7
B, C, D, H, W = 2, 3, 160, 160, 160
NCORES = 8


# ---------------------------------------------------------------- helpers
def _ap(t, offset, dims):
    """Build an AP on tensor-handle `t` at element `offset` with [step,count] dims."""
    if isinstance(t, AP):
        return AP(t.tensor, t.offset + offset, [list(d) for d in dims])
    if hasattr(t, "ap") and not hasattr(t, "shape"):
        t = t[:]
    if isinstance(t, AP):
        return AP(t.tensor, t.offset + offset, [list(d) for d in dims])
    try:
        return AP(t, offset, [list(d) for d in dims])
    except AssertionError:
        base = t[:]
        return AP(base.tensor, base.offset + offset, [list(d) for d in dims])


def _sub(ap_, offset, dims):
    """Sub-AP of an existing AP (SBUF tile view): add offset, replace free dims.

    ap_ must be a [128, free] (or [P, free]) contiguous tile AP; partition dim
    is kept, free dims replaced by `dims` at extra element `offset`.
    """
    part = ap_.ap[0]
    return AP(ap_.tensor, ap_.offset + offset, [list(part)] + [list(d) for d in dims])


# ---------------------------------------------------------------- program
def build_program(D_, H_, W_, ZS, iters, ly=16, debug=False):
    """Build the SPMD Bass program. Each core: ZS z-slices of one batch.

    Fixed partition layout: 128 = PZ(8) z-slices x PXQ(16) x-quarters.
    """
    PZ, PY, PXQ = 8, 16, 16
    assert ZS % PZ == 0 and W_ % PXQ == 0 and H_ % PY == 0 and H_ % ly == 0
    XCH = W_ // PXQ          # x positions per partition chunk (A-build pass)
    HW = H_ * W_
    NPOS = D_ * HW           # x-position count of full volume (chunk units)
    M = W_                   # voxels per partition per tile (one x-row)
    NZP = ZS // PZ           # z passes
    NYT = H_ // PY           # y tiles (main pass: 16 y-rows per tile)
    SH_VOX = ZS * HW         # voxels in shard

    nc = bass.Bass()
    flow0_e = nc.declare_dram_parameter("flow0", [SH_VOX * C], F32, isOutput=False)
    grid_e = nc.declare_dram_parameter("grid", [SH_VOX * C], F32, isOutput=False)
    out_e = nc.declare_dram_parameter("out", [SH_VOX * C], F32, isOutput=True)
    if debug:
        dbg_g = nc.declare_dram_parameter(
            "dbg_g", [128, W_ * 24], F16, isOutput=True)
        dbg_i = nc.declare_dram_parameter(
            "dbg_i", [128, W_], I32, isOutput=True)

    groups = [[0, 1, 2, 3], [4, 5, 6, 7]]

    with tile.TileContext(nc) as tc:
        frees = []

        def dram(name, shape, dtype):
            t, fr = tc.tile(shape, dtype, space="DRAM", name=name)
            frees.append(fr)
            return t

        fb = [dram(f"fbuf{i}", [SH_VOX * C], F32) for i in range(2)]
        ash = [dram(f"ashard{i}", [ZS * HW * 12], F16) for i in range(2)]
        afull = [dram(f"afull{i}", [NPOS * 12], F16) for i in range(2)]

        with (
            tc.tile_pool(name="io", bufs=3) as io_pool,
            tc.tile_pool(name="gat", bufs=3) as gat_pool,
            tc.tile_pool(name="tmp", bufs=2) as tmp_pool,
            tc.tile_pool(name="ab", bufs=3) as ab_pool,
        ):
            for k in range(-1, iters):
                asrc_t = afull[k % 2]
                fsrc_t = flow0_e if k == 0 else fb[k % 2]
                fdst_t = out_e if k == iters - 1 else fb[(k + 1) % 2]
                if k == -1:
                    fdst_t = flow0_e
                # source A viewed as [NPOS, 12] for indirect gather (coef=12)
                asrc_rows = _ap(asrc_t, 0, [(12, NPOS), (1, 12)])

                for zp in range(NZP if k >= 0 else 0):
                    z_base = zp * PZ
                    for yt in range(NYT):
                        y0 = yt * PY
                        # ---- load grid & flow tiles: [128, W*3] f32
                        # partition p = (z_loc, y_loc); contiguous (x,c) row
                        def tile_src(tens):
                            return _ap(
                                tens,
                                (z_base * HW + y0 * W_) * C,
                                [
                                    (HW * C, PZ),       # z_loc -> partition hi
                                    (W_ * C, PY),       # y_loc -> partition lo
                                    (1, W_ * C),        # x,c contiguous
                                ],
                            )

                        gl = io_pool.tile([128, M * 3], F32, tag="gl")
                        fl = io_pool.tile([128, M * 3], F32, tag="fl")
                        nc.sync.dma_start(gl[:], tile_src(grid_e))
                        nc.sync.dma_start(fl[:], tile_src(fsrc_t))

                        # ---- coords: pos = clip(g*(S-1)/2 + (S-1)/2, 0, S-1)
                        # all axes share scale since D==H==W
                        sc = (W_ - 1) / 2.0
                        pos = tmp_pool.tile([128, M * 3], F32, tag="pos")
                        nc.vector.tensor_tensor(
                            out=pos[:], in0=gl[:], in1=fl[:], op=OP.add)
                        nc.vector.tensor_scalar(
                            out=pos[:], in0=pos[:], scalar1=sc, scalar2=sc,
                            op0=OP.mult, op1=OP.add)
                        nc.vector.tensor_scalar(
                            out=pos[:], in0=pos[:], scalar1=float(W_ - 1),
                            scalar2=0.0, op0=OP.min, op1=OP.max)
                        # frac/base: base = min(floor(pos), S-2); f = pos-base
                        # floor via int cast; robust to trunc OR round-to-nearest
                        fr = tmp_pool.tile([128, M * 3], F32, tag="fr")
                        base = tmp_pool.tile([128, M * 3], F32, tag="base")
                        bi_ = tmp_pool.tile([128, M * 3], I32, tag="bi")
                        nc.vector.tensor_copy(out=bi_[:], in_=pos[:])
                        nc.vector.tensor_copy(out=base[:], in_=bi_[:])
                        nc.vector.tensor_tensor(
                            out=fr[:], in0=base[:], in1=pos[:], op=OP.is_gt)
                        nc.vector.tensor_tensor(
                            out=base[:], in0=base[:], in1=fr[:], op=OP.subtract)
                        nc.vector.tensor_scalar(
                            out=base[:], in0=base[:], scalar1=float(W_ - 2),
                            scalar2=None, op0=OP.min)
                        nc.vector.tensor_tensor(
                            out=fr[:], in0=pos[:], in1=base[:], op=OP.subtract)

                        # strided [128, M] views of axis a: offset a, stride 3
                        def ax(t_ap, a):
                            return _sub(t_ap[:], a, [(3, M)])

                        # ---- flat chunk index: bx + W*by + HW*bz
                        idxf = tmp_pool.tile([128, M], F32, tag="idxf")
                        t0 = tmp_pool.tile([128, M], F32, tag="t0")
                        nc.vector.tensor_scalar(
                            out=idxf[:], in0=ax(base, 1), scalar1=float(W_),
                            scalar2=None, op0=OP.mult)
                        nc.vector.tensor_tensor(
                            out=idxf[:], in0=idxf[:], in1=ax(base, 0), op=OP.add)
                        nc.vector.tensor_scalar(
                            out=t0[:], in0=ax(base, 2), scalar1=float(HW),
                            scalar2=None, op0=OP.mult)
                        nc.vector.tensor_tensor(
                            out=idxf[:], in0=idxf[:], in1=t0[:], op=OP.add)
                        idxi = gat_pool.tile([128, M], I32, tag="idxi")
                        nc.vector.tensor_copy(out=idxi[:], in_=idxf[:])

                        # ---- gather: 2*M instructions, 128 idx each
                        gt = gat_pool.tile([128, M * 24], F16, tag="gt")
                        for s in range(M):
                            nc.gpsimd.indirect_dma_start(
                                out=_sub(gt[:], s * 24, [(1, 24)]),
                                out_offset=None,
                                in_=asrc_rows,
                                in_offset=bass.IndirectOffsetOnAxis(
                                    ap=_sub(idxi[:], s, [(1, 1)]), axis=0),
                            )

                        if debug and k == 0 and zp == 0 and yt == 0:
                            nc.sync.dma_start(dbg_g[:], gt[:])
                            nc.sync.dma_start(dbg_i[:], idxi[:])

                        # ---- weights
                        wz0 = tmp_pool.tile([128, M], F32, tag="wz0")
                        wy0 = tmp_pool.tile([128, M], F32, tag="wy0")
                        wx0 = tmp_pool.tile([128, M], F32, tag="wx0")
                        nc.vector.tensor_scalar(
                            out=wx0[:], in0=ax(fr, 0), scalar1=-1.0, scalar2=1.0,
                            op0=OP.mult, op1=OP.add)
                        nc.vector.tensor_scalar(
                            out=wy0[:], in0=ax(fr, 1), scalar1=-1.0, scalar2=1.0,
                            op0=OP.mult, op1=OP.add)
                        nc.vector.tensor_scalar(
                            out=wz0[:], in0=ax(fr, 2), scalar1=-1.0, scalar2=1.0,
                            op0=OP.mult, op1=OP.add)

                        acc = tmp_pool.tile([128, M * 3], F32, tag="acc")
                        prod = tmp_pool.tile([128, M * 3], F32, tag="prod")
                        wtmp = tmp_pool.tile([128, M], F32, tag="wtmp")
                        first = True
                        for a_ in range(2):   # z corner (chunk slot parity)
                            for xx in range(2):
                                for b_ in range(2):
                                    # w = wz_a * wx_xx * wy_b
                                    nc.vector.tensor_tensor(
                                        out=wtmp[:],
                                        in0=(wz0[:] if a_ == 0 else ax(fr, 2)),
                                        in1=(wx0[:] if xx == 0 else ax(fr, 0)),
                                        op=OP.mult)
                                    nc.vector.tensor_tensor(
                                        out=wtmp[:], in0=wtmp[:],
                                        in1=(wy0[:] if b_ == 0 else ax(fr, 1)),
                                        op=OP.mult)
                                    goff = xx * 12 + a_ * 6 + b_ * 3
                                    gview = _sub(gt[:], goff, [(24, M), (1, 3)])
                                    wview = _sub(wtmp[:], 0, [(1, M), (0, 3)])
                                    dst = acc if first else prod
                                    nc.vector.tensor_tensor(
                                        out=dst[:], in0=gview, in1=wview,
                                        op=OP.mult)
                                    if not first:
                                        nc.vector.tensor_tensor(
                                            out=acc[:], in0=acc[:], in1=prod[:],
                                            op=OP.add)
                                    first = False

                        # ---- new flow = old flow + acc ; store
                        fo = io_pool.tile([128, M * 3], F32, tag="fo")
                        nc.vector.tensor_tensor(
                            out=fo[:], in0=fl[:], in1=acc[:], op=OP.add)
                        nc.sync.dma_start(
                            _ap(
                                fdst_t,
                                (z_base * HW + y0 * W_) * C,
                                [
                                    (HW * C, PZ),
                                    (W_ * C, PY),
                                    (1, W_ * C),
                                ],
                            ),
                            fo[:],
                        )

                # ---- A-shard build pass (skip last iteration)
                # partition = (z_loc 8, xq 16); free = (y, xch, c); per-row DMAs
                if k == iters - 1:
                    continue
                adst = ash[(k + 1) % 2]
                for zp in range(NZP):
                    z_base = zp * PZ
                    last_zp = zp == NZP - 1
                    for yt in range(H_ // ly):
                        y0 = yt * ly
                        nrows = ly + 1 if y0 + ly < H_ else ly
                        ft = ab_pool.tile([128, (ly + 1) * XCH * C], F32,
                                          tag="ft")
                        ft1 = ab_pool.tile([128, (ly + 1) * XCH * C], F32,
                                           tag="ft1")
                        for r in range(nrows):
                            nc.sync.dma_start(
                                _sub(ft[:], r * XCH * C, [(1, XCH * C)]),
                                _ap(
                                    fdst_t,
                                    (z_base * HW + (y0 + r) * W_) * C,
                                    [
                                        (HW * C, PZ),
                                        (XCH * C, PXQ),
                                        (1, XCH * C),
                                    ],
                                ),
                            )
                            # z+1 rows; final z-slice of shard: clamp (repaired
                            # after AllGather for interior shard boundaries)
                            if not last_zp:
                                nc.sync.dma_start(
                                    _sub(ft1[:], r * XCH * C, [(1, XCH * C)]),
                                    _ap(
                                        fdst_t,
                                        ((z_base + 1) * HW + (y0 + r) * W_) * C,
                                        [
                                            (HW * C, PZ),
                                            (XCH * C, PXQ),
                                            (1, XCH * C),
                                        ],
                                    ),
                                )
                            else:
                                nc.sync.dma_start(
                                    _sub(ft1[:], r * XCH * C,
                                         [(1, XCH * C)])[: (PZ - 1) * PXQ],
                                    _ap(
                                        fdst_t,
                                        ((z_base + 1) * HW + (y0 + r) * W_) * C,
                                        [
                                            (HW * C, PZ - 1),
                                            (XCH * C, PXQ),
                                            (1, XCH * C),
                                        ],
                                    ),
                                )
                                nc.sync.dma_start(
                                    _sub(ft1[:], r * XCH * C,
                                         [(1, XCH * C)])[(PZ - 1) * PXQ:],
                                    _ap(
                                        fdst_t,
                                        ((z_base + PZ - 1) * HW
                                         + (y0 + r) * W_) * C,
                                        [
                                            (HW * C, 1),
                                            (XCH * C, PXQ),
                                            (1, XCH * C),
                                        ],
                                    ),
                                )
                        if nrows == ly:  # last tile: duplicate final row
                            for t_ in (ft, ft1):
                                nc.vector.tensor_copy(
                                    out=_sub(t_[:], ly * XCH * C,
                                             [(1, XCH * C)]),
                                    in_=_sub(t_[:], (ly - 1) * XCH * C,
                                             [(1, XCH * C)]),
                                )
                        at = ab_pool.tile([128, ly * XCH * 12], F16, tag="at")
                        # at[y, x, zz, yy, c] = F[z+zz, y+yy, x, c]
                        for zz in range(2):
                            srct = ft if zz == 0 else ft1
                            for yy in range(2):
                                nc.scalar.activation(
                                    out=_sub(at[:], zz * 6 + yy * 3,
                                             [(XCH * 12, ly), (12, XCH),
                                              (1, 3)]),
                                    in_=_sub(srct[:], yy * XCH * C,
                                             [(XCH * C, ly), (3, XCH), (1, 3)]),
                                    func=mybir.ActivationFunctionType.Copy,
                                )
                        for r in range(ly):
                            nc.sync.dma_start(
                                _ap(
                                    adst,
                                    (z_base * HW + (y0 + r) * W_) * 12,
                                    [
                                        (HW * 12, PZ),
                                        (XCH * 12, PXQ),
                                        (1, XCH * 12),
                                    ],
                                ),
                                _sub(at[:], r * XCH * 12, [(1, XCH * 12)]),
                            )

                # ---- exchange: AllGather A-shards within batch group
                nc.gpsimd.collective_compute(
                    "AllGather",
                    OP.bypass,
                    replica_groups=groups,
                    ins=[adst[:]],
                    outs=[afull[(k + 1) % 2][:]],
                )
                # repair zz=1 halves of interior shard-boundary slices:
                # A[zb][..][1][yy][c] <- A[zb+1][..][0][yy][c]
                af_t = afull[(k + 1) % 2]
                nslabs = D_ // ZS
                for sb in range(nslabs - 1):
                    zb = sb * ZS + ZS - 1
                    rep = ab_pool.tile([128, (HW // 128) * 6], F16, tag="rep")
                    nc.sync.dma_start(
                        rep[:],
                        _ap(af_t, (zb + 1) * HW * 12,
                            [(12 * (HW // 128), 128), (12, HW // 128),
                             (1, 6)]),
                    )
                    nc.sync.dma_start(
                        _ap(af_t, zb * HW * 12 + 6,
                            [(12 * (HW // 128), 128), (12, HW // 128),
                             (1, 6)]),
                        rep[:],
                    )

        for fr in frees:
            fr()

    from birpatch_inline import split_excess_sync

    split_excess_sync(nc)
    return nc


# birpatch inlined as a module-level fallback (kernel.py must be self-contained)
import types

_bp = types.ModuleType("birpatch_inline")
_bp_code = '''
import concourse.mybir as mybir


def split_excess_sync(nc, maxw=1, maxu=16):
    for bb in nc.main_func.blocks:
        il = bb.instructions
        i = 0
        while i < len(il):
            inst = il[i]
            si = getattr(inst, "sync_info", None)
            if si is None:
                i += 1
                continue
            waits = list(si.on_wait or [])
            if len(waits) > maxw:
                extra, keep = waits[:-maxw], waits[-maxw:]
                si.on_wait = keep
                pos = i
                for j in range(0, len(extra), maxw):
                    chunk = extra[j:j + maxw]
                    nop = nc.engines[inst.engine].nop(nofuse=True).ins
                    _remove_from_blocks(nc, nop)
                    nop.sync_info = mybir.SyncInfo(on_wait=chunk, on_update=[])
                    il.insert(pos, nop)
                    pos += 1
                    i += 1
            i += 1


def _remove_from_blocks(nc, inst):
    for bb in nc.main_func.blocks:
        il = bb.instructions
        for k in range(len(il) - 1, -1, -1):
            if il[k] is inst:
                del il[k]
                return
    raise RuntimeError("nop not found")
'''
exec(_bp_code, _bp.__dict__)
sys.modules["birpatch_inline"] = _bp


# ---------------------------------------------------------------- host side
_CACHE = {}


def _get_program(D_, H_, W_, ZS, iters, ly):
    key = (D_, H_, W_, ZS, iters, ly)
    if key not in _CACHE:
        _CACHE[key] = build_program(D_, H_, W_, ZS, iters, ly)
    return _CACHE[key]


def run(velocity, sample_grid, D_=D, H_=H, W_=W, iters=TIME_STEP, ly=16,
        trace=False):
    B_ = velocity.shape[0]
    ncores = 8
    n_slab = ncores // B_
    ZS = D_ // n_slab
    nc = _get_program(D_, H_, W_, ZS, iters, ly)

    f0 = np.transpose(velocity, (0, 2, 3, 4, 1)).astype(np.float32) / (2.0 ** 7)

    in_maps = []
    for i in range(ncores):
        b = i // n_slab
        zsl = slice((i % n_slab) * ZS, (i % n_slab) * ZS + ZS)
        in_maps.append({
            "flow0": np.ascontiguousarray(f0[b, zsl]).ravel(),
            "grid": np.ascontiguousarray(
                sample_grid[b, zsl]).astype(np.float32).ravel(),
        })
    res = run_bass_kernel_spmd(nc, in_maps, list(range(ncores)), trace=trace)
    outs = res.results
    full = np.empty((B_, D_, H_, W_, C), np.float32)
    for i in range(ncores):
        b = i // n_slab
        zsl = slice((i % n_slab) * ZS, (i % n_slab) * ZS + ZS)
        full[b, zsl] = outs[i]["out"].reshape(ZS, H_, W_, C)
    out = np.transpose(full, (0, 4, 1, 2, 3))
    return np.ascontiguousarray(out), res


def kernel(velocity, sample_grid):
    out, _ = run(np.asarray(velocity), np.asarray(sample_grid))
    return out
